# revision 17
# baseline (speedup 1.0000x reference)
"""NeuralMemory fast-weight recurrence on 8 Trainium2 NeuronCores — v4.

Sharding: 8-way tensor-parallel over memory dim M=2048 (MS=256/core).
One fp8 AllReduce per chunk whose payload is c*(pred partial + qb1/8) - x/8,
so the AR output IS dpred directly.

v4 changes vs v3:
- z-linearization: z(x_{j+1}, P_{j+1}) = z(x_{j+1}, P_j) + s*(x_{j+1}x_j^T)@dz_j.
  The 32-matmul layer-1 pass for the next chunk's pred runs in the AR shadow
  under the OLD weights; the critical section applies a 4-matmul correction.
- gW0 and the q0t update move into the AR shadow entirely (only the next
  shadow's zOLD pass consumes them).
- Transposes emitted as plain matmuls against a stationary slice streaming
  the identity (out = src^T @ I): pipelines at the ~100ns matmul issue rate
  instead of ~400ns serial transpose-mode ops.
- Shadow per chunk now: gW0 + out-forward + gW1n + G gram + zOLD — roughly
  balancing the ~45us AllReduce.
"""
import numpy as np
import concourse.bacc as bacc
import concourse.mybir as mybir
import concourse.tile as tile
from concourse.bass_utils import run_bass_kernel_spmd

BF = mybir.dt.bfloat16
FP8 = mybir.dt.float8e4
F32 = mybir.dt.float32
AF = mybir.ActivationFunctionType
ALU = mybir.AluOpType

NCORES = 8
B, L, D, M = 2, 2048, 2048, 2048
C = 128                 # reference CHUNK
NCH = L // C            # 16 chunks
T = B * C               # 256 tokens per chunk
MS = M // NCORES        # 256 per-core memory slice
KD = D // 128           # 16 tiles over D
KT = T // 128           # 2 tiles over tokens
KM = MS // 128          # 2 tiles over m_s
NN = D // 512           # 4 N-chunks of 512 over D
LR_MEMORY = 0.01

# scal columns: [c, cn, f, negs, negs0x8, negs8, negs0p, negs0G]
SC_C, SC_CN, SC_F, SC_NEGS, SC_NEGS0, SC_NEGS8, SC_NEGS0P, SC_NEGS0G = range(8)


def build(ar=True):
    nc = bacc.Bacc("TRN2", target_bir_lowering=False, num_devices=NCORES)
    xr8_in = nc.dram_tensor("xr8", [NCH, T, D], BF, kind="ExternalInput")
    xt_in = nc.dram_tensor("xt", [NCH, D, T], BF, kind="ExternalInput")
    w0t_in = nc.dram_tensor("w0t", [D, MS], F32, kind="ExternalInput")
    w1t_in = nc.dram_tensor("w1t", [MS, D], F32, kind="ExternalInput")
    w1n_in = nc.dram_tensor("w1n", [D, MS], F32, kind="ExternalInput")
    b0c_in = nc.dram_tensor("b0c", [128, KM], F32, kind="ExternalInput")
    b1d8_in = nc.dram_tensor("b1d8", [1, D], F32, kind="ExternalInput")
    scal_in = nc.dram_tensor("scal", [1, NCH * 8], F32, kind="ExternalInput")
    ident_in = nc.dram_tensor("ident", [128, 128], F32, kind="ExternalInput")
    outq = nc.dram_tensor("outq", [NCH, T, D], BF, kind="ExternalOutput")
    b1out = nc.dram_tensor("b1out", [NCH, D], F32, kind="ExternalOutput")

    with tile.TileContext(nc) as tc:
        with (
            tc.tile_pool(name="wp", bufs=1) as wp,            # persistent
            tc.tile_pool(name="xp", bufs=4) as xp,            # x streams
            tc.tile_pool(name="ap", bufs=2) as ap,            # loop-carried acts
            tc.tile_pool(name="tp", bufs=2) as tp,            # per-iter temps
            tc.tile_pool(name="psA", bufs=2, space="PSUM") as psA,   # [128,512]
            tc.tile_pool(name="psB", bufs=2, space="PSUM") as psB,   # [128,512]
            tc.tile_pool(name="psZ", bufs=1, space="PSUM") as psZ,   # [128,512]
            tc.tile_pool(name="psT", bufs=2, space="PSUM") as psT,   # [128,1024] bf
            tc.tile_pool(name="psD", bufs=1, space="PSUM") as psD,   # small
            tc.tile_pool(name="dr", bufs=2, space="DRAM") as dr,
        ):
            # ---------------- persistent state ----------------
            q0t = wp.tile([128, KD * MS], BF, name="q0t")   # [d,m] d-tile i @ i*MS
            q1t = wp.tile([128, KM * D], BF, name="q1t")    # [m,d] m-tile k @ k*D
            q1n = wp.tile([128, KD * MS], BF, name="q1n")   # [d,m] d-tile i @ i*MS
            bp0 = wp.tile([128, KM], F32, name="bp0")       # P-space b0 columns
            bk1 = wp.tile([1, D], F32, name="bk1")          # Q-space b1/8 row
            ident = wp.tile([128, 128], BF, name="ident")
            ones_col = wp.tile([128, 1], BF, name="ones_col")
            ones_row = wp.tile([1, 128], BF, name="ones_row")
            junk = wp.tile([1, 1], BF, name="junk")
            bk1bf = wp.tile([1, D], BF, name="bk1bf")
            scal = wp.tile([1, NCH * 8], F32, name="scal")
            cbc = wp.tile([128, NCH], F32, name="cbc")
            cnbc = wp.tile([128, NCH], F32, name="cnbc")
            fbc = wp.tile([128, NCH], F32, name="fbc")
            negsbc = wp.tile([128, NCH], F32, name="negsbc")
            negs0bc = wp.tile([128, NCH], F32, name="negs0bc")
            negs0pbc = wp.tile([128, NCH], F32, name="negs0pbc")
            negs0Gbc = wp.tile([128, NCH], F32, name="negs0Gbc")

            nc.gpsimd.dma_start(
                q0t[:].rearrange("p (i m) -> p i m", m=MS),
                w0t_in[:].rearrange("(i p) m -> p i m", p=128))
            nc.gpsimd.dma_start(
                q1t[:].rearrange("p (k d) -> p k d", d=D),
                w1t_in[:].rearrange("(k p) d -> p k d", p=128))
            nc.gpsimd.dma_start(
                q1n[:].rearrange("p (i m) -> p i m", m=MS),
                w1n_in[:].rearrange("(i p) m -> p i m", p=128))
            nc.sync.dma_start(bp0[:], b0c_in[:])
            nc.sync.dma_start(bk1[:], b1d8_in[:])
            nc.gpsimd.dma_start(ident[:], ident_in[:])
            nc.vector.memset(ones_col[:], 1.0)
            nc.vector.memset(ones_row[:], 1.0)
            nc.scalar.copy(bk1bf[:], bk1[:])
            nc.sync.dma_start(scal[:], scal_in[:])
            for j in range(NCH):
                for dst, col in ((cbc, SC_C), (cnbc, SC_CN), (fbc, SC_F),
                                 (negsbc, SC_NEGS), (negs0bc, SC_NEGS0),
                                 (negs0pbc, SC_NEGS0P), (negs0Gbc, SC_NEGS0G)):
                    nc.gpsimd.partition_broadcast(
                        dst[:, j:j + 1], scal[0:1, j * 8 + col:j * 8 + col + 1])

            # ---------------- helpers ----------------
            def load_x(j):
                xb = xp.tile([128, KT * D], BF, name=f"xb{j}", tag="xb")
                for k in range(KT):
                    nc.sync.dma_start(xb[:, k * D:(k + 1) * D],
                                      xr8_in[j, k * 128:(k + 1) * 128, :])
                xT = xp.tile([128, KD * T], BF, name=f"xT{j}", tag="xT")
                nc.sync.dma_start(
                    xT[:].rearrange("p (i t) -> p i t", t=T),
                    xt_in[j].rearrange("(i p) t -> p i t", p=128))
                return xb, xT

            def mmT(pt_slice, src_slice):
                nc.tensor.transpose(pt_slice, src_slice, ident[:])

            def mm1_full(xT, j, pfx, stop=True, zold_tag=False):
                """layer-1 q-space psum [m_s, KM*T] packed, under current q0t."""
                pool = psZ if zold_tag else psB
                pt = pool.tile([128, KM * T], F32, name=f"z{pfx}_{j}",
                               tag="psZ" if zold_tag else "psB")
                for a in range(KM):
                    for i in range(KD):
                        nc.tensor.matmul(
                            pt[:, a * T:(a + 1) * T],
                            q0t[:, i * MS + a * 128:i * MS + (a + 1) * 128],
                            xT[:, i * T:(i + 1) * T],
                            start=(i == 0), stop=(stop and i == KD - 1))
                return pt

            def silu_evac(zpt, j, cnc, want_dsilu, pfx):
                hT = (ap if want_dsilu else tp).tile(
                    [128, KM * T], BF, name=f"h{pfx}_{j}", tag=f"h{pfx}")
                for a in range(KM):
                    nc.scalar.activation(hT[:, a * T:(a + 1) * T],
                                         zpt[:, a * T:(a + 1) * T],
                                         AF.Silu, bias=bp0[:, a:a + 1],
                                         scale=cnc)
                if not want_dsilu:
                    return hT, None, None
                hpT = ap.tile([128, KM * T], BF, name=f"hp{pfx}_{j}", tag="hp1")

                def emit_dsilu():
                    for a in range(KM):
                        nc.scalar.activation(hpT[:, a * T:(a + 1) * T],
                                             zpt[:, a * T:(a + 1) * T],
                                             AF.Derivative_silu,
                                             bias=bp0[:, a:a + 1], scale=cnc)
                return hT, hpT, emit_dsilu

            def mm2_R(hT, j, pfx, evac, with_bias=False):
                """R-form layer 2 partial: psums [t',512], evac(k, n, pt) inline."""
                for k in range(KT):
                    for n in range(NN):
                        pt = psA.tile([128, 512], F32,
                                      name=f"p{pfx}_{j}_{k}_{n}", tag="psA")
                        for a in range(KM):
                            nc.tensor.matmul(
                                pt[:],
                                hT[:, a * T + k * 128:a * T + (k + 1) * 128],
                                q1t[:, a * D + n * 512:a * D + (n + 1) * 512],
                                start=(a == 0),
                                stop=(a == KM - 1 and not with_bias))
                        if with_bias:
                            nc.tensor.matmul(
                                pt[:], ones_row[:],
                                bk1bf[0:1, n * 512:(n + 1) * 512],
                                start=False, stop=True)
                        evac(k, n, pt)

            def evac_pred(k, n, pt, dst, csc, xb):
                # dst = c*psum - x/8 -> AR sums to dpred
                sl = slice(k * D + n * 512, k * D + (n + 1) * 512)
                if n % 2 == 0:
                    nc.vector.scalar_tensor_tensor(dst[:, sl], pt[:], csc,
                                                   xb[:, sl], ALU.mult,
                                                   ALU.subtract)
                else:
                    gqp = tp.tile([128, 512], BF, name=f"gqp{id(pt)}", tag="gqp")
                    nc.scalar.mul(gqp[:], pt[:], csc)
                    nc.gpsimd.tensor_tensor(dst[:, sl], gqp[:], xb[:, sl],
                                            ALU.subtract)

            def send_ar(predp, j):
                arin = dr.tile([T, D], FP8, name=f"arin{j}", tag="arin")
                for k in range(KT):
                    for h in range(2):
                        eng = nc.sync if h == 0 else nc.scalar
                        eng.dma_start(
                            arin[k * 128:(k + 1) * 128,
                                 h * 1024:(h + 1) * 1024],
                            predp[:, k * D + h * 1024:k * D + (h + 1) * 1024])
                if ar:
                    arout = dr.tile([T, D], FP8, name=f"arout{j}", tag="arout",
                                    addr_space="Shared")
                    nc.gpsimd.collective_compute(
                        "AllReduce", ALU.add, replica_groups=[list(range(NCORES))],
                        ins=[arin.opt()], outs=[arout.opt()])
                else:
                    arout = arin
                return arout

            def h1R_T(h1T, j):
                """batched [t,m] form of h via matmul-transposes."""
                h1R = ap.tile([128, KT * MS], BF, name=f"h1R{j}", tag="h1R")
                pt = psT.tile([128, 1024], BF, name=f"Th{j}", tag="psT")
                for k in range(KT):
                    for a in range(KM):
                        mmT(pt[:, (k * KM + a) * 128:(k * KM + a + 1) * 128],
                            h1T[:, a * T + k * 128:a * T + (k + 1) * 128])
                nc.scalar.copy(h1R[:], pt[:, 0:KT * MS])
                return h1R

            def gram_G(xTa, xTb, jn):
                """Gs = negs0G_{jn} * (x_a x_b^T): [t(a) 2-tiles, t'(b)] bf16."""
                Gs = ap.tile([128, KT * T], BF, name=f"Gs{jn}", tag="Gs")
                pt = psB.tile([128, KT * T], F32, name=f"G{jn}", tag="psB")
                for k in range(KT):
                    for i in range(KD):
                        nc.tensor.matmul(
                            pt[:, k * T:(k + 1) * T],
                            xTa[:, i * T + k * 128:i * T + (k + 1) * 128],
                            xTb[:, i * T:(i + 1) * T],
                            start=(i == 0), stop=(i == KD - 1))
                nc.vector.tensor_scalar_mul(Gs[:], pt[:], negs0Gbc[:, jn:jn + 1])
                return Gs

            # ---------------- prologue: chunk 0 pred under P_0 ----------------
            xb_c, xT_c = load_x(0)
            xb_n, xT_n = load_x(1)
            c0 = cbc[:, 0:1]
            z0 = mm1_full(xT_c, 0, "1")
            h1T_c, hp1T_c, dsilu0 = silu_evac(z0, 0, c0, True, "1")
            predp = ap.tile([128, KT * D], FP8, name="predp0", tag="predp")
            mm2_R(h1T_c, 0, "p",
                  lambda k, n, pt: evac_pred(k, n, pt, predp, c0, xb_c),
                  with_bias=True)
            arout = send_ar(predp, 0)
            dsilu0()
            # ---- shadow of AR_0 ----
            h1R_c = h1R_T(h1T_c, 0)
            xb_p, xT_p = load_x(2)
            Gs_c = gram_G(xT_c, xT_n, 0)          # for crit_0's correction
            zold = mm1_full(xT_n, 1, "z", stop=False, zold_tag=True)

            # ---------------- main loop ----------------
            for j in range(NCH):
                last = (j == NCH - 1)
                cnj = cnbc[:, j:j + 1]
                fj = fbc[:, j:j + 1]
                ngj = negsbc[:, j:j + 1]
                ng0j = negs0bc[:, j:j + 1]

                # ======== critical section: consume AR_j ========
                dpR = tp.tile([128, KT * D], BF, name=f"dpR{j}", tag="dpR")
                for h in range(2):          # d-low halves first for both k
                    for k in range(KT):
                        nc.gpsimd.dma_start(
                            dpR[:, k * D + h * 1024:k * D + (h + 1) * 1024],
                            arout[k * 128:(k + 1) * 128, h * 1024:(h + 1) * 1024])
                nc.scalar.activation(junk[:], scal[0:1, 0:1], AF.Silu)

                # dpT via batched transposes (4 groups of 8)
                dpT = tp.tile([128, KD * T], BF, name=f"dpT{j}", tag="dpT")
                for g in range(4):
                    pt = psT.tile([128, 1024], BF, name=f"TdP{j}_{g}", tag="psT")
                    for x in range(8):
                        s = g * 8 + x
                        i, k = s // KT, s % KT
                        mmT(pt[:, x * 128:(x + 1) * 128],
                            dpR[:, k * D + i * 128:k * D + (i + 1) * 128])
                    eng = nc.vector if g % 2 else nc.scalar
                    if eng is nc.scalar:
                        nc.scalar.copy(dpT[:, g * 1024:(g + 1) * 1024], pt[:])
                    else:
                        eng.tensor_copy(dpT[:, g * 1024:(g + 1) * 1024], pt[:])

                # dhT = W1 dpred^T [m,t] ; dzT = dhT * hp
                dzT = tp.tile([128, KM * T], BF, name=f"dzT{j}", tag="dzT")
                pt = psB.tile([128, KM * T], F32, name=f"dh{j}", tag="psB")
                for a in range(KM):
                    for i in range(KD):
                        nc.tensor.matmul(
                            pt[:, a * T:(a + 1) * T],
                            q1n[:, i * MS + a * 128:i * MS + (a + 1) * 128],
                            dpT[:, i * T:(i + 1) * T],
                            start=(i == 0), stop=(i == KD - 1))
                for a in range(KM):
                    sl = slice(a * T, (a + 1) * T)
                    nc.vector.tensor_tensor(dzT[:, sl], pt[:, sl], hp1T_c[:, sl],
                                            ALU.mult)
                # dzR via batched matmul-transpose (one group of 4)
                dzR = tp.tile([128, KT * MS], BF, name=f"dzR{j}", tag="dzR")
                pt = psT.tile([128, 1024], BF, name=f"Tdz{j}", tag="psT")
                for k in range(KT):
                    for a in range(KM):
                        mmT(pt[:, (k * KM + a) * 128:(k * KM + a + 1) * 128],
                            dzT[:, a * T + k * 128:a * T + (k + 1) * 128])
                nc.vector.tensor_copy(dzR[:], pt[:, 0:KT * MS])

                # gb0 -> bp0 ; gb1 -> bk1 (both needed before silu / mm2 bias)
                nc.vector.tensor_scalar_mul(bp0[:], bp0[:], fj)
                ng0pj = negs0pbc[:, j:j + 1]
                gb0r = tp.tile([128, KM], F32, name=f"gb0_{j}", tag="gb0")
                nc.vector.reduce_sum(gb0r[:],
                                     dzT[:].rearrange("p (a t) -> p a t", t=T),
                                     axis=mybir.AxisListType.X)
                nc.vector.scalar_tensor_tensor(bp0[:], gb0r[:], ng0pj,
                                               bp0[:], ALU.mult, ALU.add)
                if not last:
                    # correction: zold += Gs^T-contract dz ; then silu
                    for a in range(KM):
                        for k in range(KT):
                            nc.tensor.matmul(
                                zold[:, a * T:(a + 1) * T],
                                dzR[:, k * MS + a * 128:k * MS + (a + 1) * 128],
                                Gs_c[:, k * T:(k + 1) * T],
                                start=False, stop=(k == KT - 1))
                    h1T_n, hp1T_n, dsilu_n = silu_evac(zold, j + 1, cnj, True, "1")
                else:
                    h1T_n = hp1T_n = dsilu_n = None
                # gW1 -> q1t update (PE) while dzR evacuates
                for a in range(KM):
                    for n in range(NN):
                        pt = psA.tile([128, 512], F32, name=f"g1_{j}_{a}_{n}",
                                      tag="psA")
                        for k in range(KT):
                            nc.tensor.matmul(
                                pt[:],
                                h1R_c[:, k * MS + a * 128:k * MS + (a + 1) * 128],
                                dpR[:, k * D + n * 512:k * D + (n + 1) * 512],
                                start=(k == 0), stop=(k == KT - 1))
                        sl = slice(a * D + n * 512, a * D + (n + 1) * 512)
                        if n % 2 == 0:
                            nc.vector.scalar_tensor_tensor(q1t[:, sl], pt[:], ngj,
                                                           q1t[:, sl], ALU.mult,
                                                           ALU.add)
                        else:
                            gq = tp.tile([128, 512], BF, name=f"gq1_{j}_{a}_{n}",
                                         tag="gq")
                            nc.scalar.mul(gq[:], pt[:], ngj)
                            nc.gpsimd.tensor_tensor(q1t[:, sl], q1t[:, sl], gq[:],
                                                    ALU.add)

                for n in range(NN):
                    pt = psD.tile([1, 512], F32, name=f"gb1_{j}_{n}", tag="psDr")
                    for k in range(KT):
                        nc.tensor.matmul(
                            pt[:], ones_col[:],
                            dpR[:, k * D + n * 512:k * D + (n + 1) * 512],
                            start=(k == 0), stop=(k == KT - 1))
                    sl = slice(n * 512, (n + 1) * 512)
                    gqr = tp.tile([1, 512], F32, name=f"gqr{j}_{n}", tag="gqr")
                    nc.scalar.mul(gqr[:], pt[:],
                                  scal[0:1, j * 8 + SC_NEGS8:j * 8 + SC_NEGS8 + 1])
                    nc.gpsimd.tensor_tensor(bk1[0:1, sl], bk1[0:1, sl], gqr[:],
                                            ALU.add)
                nc.sync.dma_start(b1out[j:j + 1, :], bk1[:])
                nc.scalar.copy(bk1bf[:], bk1[:])

                if not last:
                    predp = ap.tile([128, KT * D], FP8, name=f"predp{j + 1}",
                                    tag="predp")
                    pp = predp
                    xbn = xb_n
                    mm2_R(h1T_n, j + 1, "p",
                          lambda k, n, pt: evac_pred(k, n, pt, pp, cnj, xbn),
                          with_bias=True)
                    arout = send_ar(predp, j + 1)
                    dsilu_n()

                # ======== shadow of AR_{j+1} ========
                # gW0 -> q0t update (paired psums; consumed by zOLD below)
                for i2 in range(KD // 2):
                    pt = psA.tile([128, 512], F32, name=f"g0_{j}_{i2}", tag="psA")
                    for ii in range(2):
                        i = i2 * 2 + ii
                        for k in range(KT):
                            nc.tensor.matmul(
                                pt[:, ii * MS:(ii + 1) * MS],
                                xb_c[:, k * D + i * 128:k * D + (i + 1) * 128],
                                dzR[:, k * MS:(k + 1) * MS],
                                start=(k == 0), stop=(k == KT - 1))
                    sl = slice(i2 * 512, (i2 + 1) * 512)
                    nc.vector.scalar_tensor_tensor(q0t[:, sl], pt[:], ng0j,
                                                   q0t[:, sl], ALU.mult, ALU.add)

                # out_j forward under P_{j+1} (unscaled Q-space partial)
                zo = mm1_full(xT_c, j, "2")
                h2T, _, _ = silu_evac(zo, j, cnj, False, "2")
                outsb = tp.tile([128, KT * D], BF, name=f"o{j}", tag="outsb")

                def evac_out(k, n, pt, dst=outsb):
                    sl = slice(k * D + n * 512, k * D + (n + 1) * 512)
                    if n % 2 == 0:
                        nc.vector.tensor_copy(dst[:, sl], pt[:])
                    else:
                        nc.scalar.copy(dst[:, sl], pt[:])

                mm2_R(h2T, j, "o", evac_out)
                for k in range(KT):
                    nc.sync.dma_start(outq[j, k * 128:(k + 1) * 128, :],
                                      outsb[:, k * D:(k + 1) * D])

                if not last:
                    # gW1n -> q1n update (vector-only evac: gpsimd may be
                    # blocked on the AR wait)
                    for i2 in range(KD // 2):
                        pt = psA.tile([128, 512], F32, name=f"g1n_{j}_{i2}",
                                      tag="psA")
                        for ii in range(2):
                            i = i2 * 2 + ii
                            for k in range(KT):
                                nc.tensor.matmul(
                                    pt[:, ii * MS:(ii + 1) * MS],
                                    dpR[:, k * D + i * 128:k * D + (i + 1) * 128],
                                    h1R_c[:, k * MS:(k + 1) * MS],
                                    start=(k == 0), stop=(k == KT - 1))
                        sl = slice(i2 * 512, (i2 + 1) * 512)
                        nc.vector.scalar_tensor_tensor(q1n[:, sl], pt[:], ngj,
                                                       q1n[:, sl], ALU.mult,
                                                       ALU.add)

                    h1R_n = h1R_T(h1T_n, j + 1)
                    if j + 2 < NCH:
                        # prepare next crit's pred basis: G gram + zOLD pass
                        Gs_n = gram_G(xT_n, xT_p, j + 1)
                        zold = mm1_full(xT_p, j + 2, "z", stop=False, zold_tag=True)
                        xb_f, xT_f = load_x(j + 3) if j + 3 < NCH else (None, None)
                        xb_c, xT_c = xb_n, xT_n
                        xb_n, xT_n = xb_p, xT_p
                        if xb_f is not None:
                            xb_p, xT_p = xb_f, xT_f
                        Gs_c = Gs_n
                    else:
                        xb_c, xT_c = xb_n, xT_n
                    h1T_c, hp1T_c, h1R_c = h1T_n, hp1T_n, h1R_n
    nc.compile()
    return nc


_NC_CACHE = None


def _get_nc():
    global _NC_CACHE
    if _NC_CACHE is None:
        _NC_CACHE = build()
    return _NC_CACHE


def _sigmoid(v):
    return 1.0 / (1.0 + np.exp(-v))


def host_prep(x, W0, b0, W1, b1, lr_w, lr_b, fg_w, fg_b):
    """Host-side: layouts, pre-transposed x, gate scalar schedule."""
    import ml_dtypes
    bf16 = ml_dtypes.bfloat16
    x = np.asarray(x, np.float32)
    # chunk layouts: xr8 [NCH, T, D] = x/8, xt [NCH, D, T] (token t = b*C + c)
    xch = np.transpose(x.reshape(B, NCH, C, D), (1, 0, 2, 3)).reshape(NCH, T, D)
    xr8 = np.ascontiguousarray(xch / 8.0).astype(bf16)
    xt = np.ascontiguousarray(np.transpose(xch, (0, 2, 1))).astype(bf16)

    lr_wv = np.asarray(lr_w, np.float32)[0]
    fg_wv = np.asarray(fg_w, np.float32)[0]
    lr_bv = float(np.asarray(lr_b, np.float32).reshape(-1)[0])
    fg_bv = float(np.asarray(fg_b, np.float32).reshape(-1)[0])
    scal = np.zeros((NCH, 8), np.float32)
    c = 1.0
    for j in range(NCH):
        ch = xch[j]                                   # (T, D)
        lsum = _sigmoid(ch @ lr_wv + lr_bv).sum()
        fparts = _sigmoid(ch.reshape(B, C, D).mean(axis=1) @ fg_wv + fg_bv)
        f = float(fparts.mean())
        cn = c * f
        negs = -LR_MEMORY * 2.0 * float(lsum) / (T * T * D) / cn
        # negs0 x8 (gW0 streams x/8); negs0G = negs*c (zOLD correction, x
        # unscaled in the gram matrix)
        scal[j] = [c, cn, f, negs, negs * c * 8.0, negs / 8.0, negs * c * cn,
                   negs * c]
        c = cn
    return xr8, xt, scal


def make_in_maps(x, W0, b0, W1, b1, lr_w, lr_b, fg_w, fg_b):
    xr8, xt, scal = host_prep(x, W0, b0, W1, b1, lr_w, lr_b, fg_w, fg_b)
    W0 = np.asarray(W0, np.float32)
    W1 = np.asarray(W1, np.float32)
    b0v = np.asarray(b0, np.float32)
    b1v = np.asarray(b1, np.float32)
    ident = np.eye(128, dtype=np.float32)
    in_maps = []
    for s in range(NCORES):
        sl = slice(s * MS, (s + 1) * MS)
        b0c = np.ascontiguousarray(b0v[sl].reshape(KM, 128).T)   # [128, KM]
        in_maps.append({
            "xr8": xr8,
            "xt": xt,
            "w0t": np.ascontiguousarray(W0[sl, :].T),
            "w1t": np.ascontiguousarray(W1[:, sl].T),
            "w1n": np.ascontiguousarray(W1[:, sl]),
            "b0c": b0c,
            "b1d8": np.ascontiguousarray((b1v / 8.0).reshape(1, D)),
            "scal": np.ascontiguousarray(scal.reshape(1, NCH * 8)),
            "ident": ident,
        })
    return in_maps


def run(inputs, **kw):
    nc = _get_nc()
    in_maps = make_in_maps(**inputs)
    res = run_bass_kernel_spmd(nc, in_maps, core_ids=list(range(NCORES)), **kw)
    scal = in_maps[0]["scal"].reshape(NCH, 8)
    outq = np.zeros((NCH, T, D), np.float32)
    for r in res.results:
        outq += np.asarray(r["outq"], dtype=np.float32)
    b1rows = res.results[0]["b1out"]                  # [NCH, D] = qb1/8 rows
    cn = scal[:, SC_CN].reshape(NCH, 1, 1)
    outq = cn * outq + (cn * 8.0) * b1rows.reshape(NCH, 1, D)
    out = np.ascontiguousarray(
        np.transpose(outq.reshape(NCH, B, C, D), (1, 0, 2, 3))).reshape(B, L, D)
    return out, res


def kernel(**inputs) -> np.ndarray:
    out, _ = run(inputs)
    return out


# revision 19
# speedup vs baseline: 1.4117x; 1.4117x over previous
"""NeuralMemory fast-weight recurrence on 8 Trainium2 NeuronCores — v4.

Sharding: 8-way tensor-parallel over memory dim M=2048 (MS=256/core).
One fp8 AllReduce per chunk whose payload is c*(pred partial + qb1/8) - x/8,
so the AR output IS dpred directly.

v4 changes vs v3:
- z-linearization: z(x_{j+1}, P_{j+1}) = z(x_{j+1}, P_j) + s*(x_{j+1}x_j^T)@dz_j.
  The 32-matmul layer-1 pass for the next chunk's pred runs in the AR shadow
  under the OLD weights; the critical section applies a 4-matmul correction.
- gW0 and the q0t update move into the AR shadow entirely (only the next
  shadow's zOLD pass consumes them).
- Transposes emitted as plain matmuls against a stationary slice streaming
  the identity (out = src^T @ I): pipelines at the ~100ns matmul issue rate
  instead of ~400ns serial transpose-mode ops.
- Shadow per chunk now: gW0 + out-forward + gW1n + G gram + zOLD — roughly
  balancing the ~45us AllReduce.
"""
import numpy as np
import concourse.bacc as bacc
import concourse.mybir as mybir
import concourse.tile as tile
from concourse.bass_utils import run_bass_kernel_spmd

BF = mybir.dt.bfloat16
FP8 = mybir.dt.float8e4
F32 = mybir.dt.float32
AF = mybir.ActivationFunctionType
ALU = mybir.AluOpType

NCORES = 8
B, L, D, M = 2, 2048, 2048, 2048
C = 128                 # reference CHUNK
NCH = L // C            # 16 chunks
T = B * C               # 256 tokens per chunk
MS = M // NCORES        # 256 per-core memory slice
KD = D // 128           # 16 tiles over D
KT = T // 128           # 2 tiles over tokens
KM = MS // 128          # 2 tiles over m_s
NN = D // 512           # 4 N-chunks of 512 over D
LR_MEMORY = 0.01

# scal columns: [c, cn, f, negs, negs0x8, negs8, negs0p, negs0G]
SC_C, SC_CN, SC_F, SC_NEGS, SC_NEGS0, SC_NEGS8, SC_NEGS0P, SC_NEGS0G = range(8)


def build(ar=True):
    nc = bacc.Bacc("TRN2", target_bir_lowering=False, num_devices=NCORES)
    xr8_in = nc.dram_tensor("xr8", [NCH, T, D], BF, kind="ExternalInput")
    xt_in = nc.dram_tensor("xt", [NCH, D, T], BF, kind="ExternalInput")
    w0t_in = nc.dram_tensor("w0t", [D, MS], F32, kind="ExternalInput")
    w1t_in = nc.dram_tensor("w1t", [MS, D], F32, kind="ExternalInput")
    w1n_in = nc.dram_tensor("w1n", [D, MS], F32, kind="ExternalInput")
    b0c_in = nc.dram_tensor("b0c", [128, KM], F32, kind="ExternalInput")
    b1d8_in = nc.dram_tensor("b1d8", [1, D], F32, kind="ExternalInput")
    scal_in = nc.dram_tensor("scal", [1, NCH * 8], F32, kind="ExternalInput")
    ident_in = nc.dram_tensor("ident", [128, 128], F32, kind="ExternalInput")
    outq = nc.dram_tensor("outq", [NCH, T, D], BF, kind="ExternalOutput")
    b1out = nc.dram_tensor("b1out", [NCH, D], F32, kind="ExternalOutput")

    with tile.TileContext(nc) as tc:
        with (
            tc.tile_pool(name="wp", bufs=1) as wp,            # persistent
            tc.tile_pool(name="xp", bufs=4) as xp,            # x streams
            tc.tile_pool(name="ap", bufs=2) as ap,            # loop-carried acts
            tc.tile_pool(name="tp", bufs=2) as tp,            # per-iter temps
            tc.tile_pool(name="psA", bufs=3, space="PSUM") as psA,   # [128,512]
            tc.tile_pool(name="psB", bufs=1, space="PSUM") as psB,   # [128,512]
            tc.tile_pool(name="psZ", bufs=1, space="PSUM") as psZ,   # [128,512]
            tc.tile_pool(name="psT", bufs=2, space="PSUM") as psT,   # [128,1024] bf
            tc.tile_pool(name="psD", bufs=1, space="PSUM") as psD,   # small
            tc.tile_pool(name="dr", bufs=2, space="DRAM") as dr,
        ):
            # ---------------- persistent state ----------------
            q0t = wp.tile([128, KD * MS], BF, name="q0t")   # [d,m] d-tile i @ i*MS
            q1t = wp.tile([128, KM * D], BF, name="q1t")    # [m,d] m-tile k @ k*D
            q1n = wp.tile([128, KD * MS], BF, name="q1n")   # [d,m] d-tile i @ i*MS
            bp0 = wp.tile([128, KM], F32, name="bp0")       # P-space b0 columns
            bk1 = wp.tile([1, D], F32, name="bk1")          # Q-space b1/8 row
            ident = wp.tile([128, 128], BF, name="ident")
            ones_col = wp.tile([128, 1], BF, name="ones_col")
            ones_row = wp.tile([1, 128], BF, name="ones_row")
            junk = wp.tile([1, 1], BF, name="junk")
            bk1bf = wp.tile([1, D], BF, name="bk1bf")
            scal = wp.tile([1, NCH * 8], F32, name="scal")
            cbc = wp.tile([128, NCH], F32, name="cbc")
            cnbc = wp.tile([128, NCH], F32, name="cnbc")
            fbc = wp.tile([128, NCH], F32, name="fbc")
            negsbc = wp.tile([128, NCH], F32, name="negsbc")
            negs0bc = wp.tile([128, NCH], F32, name="negs0bc")
            negs0pbc = wp.tile([128, NCH], F32, name="negs0pbc")
            negs0Gbc = wp.tile([128, NCH], F32, name="negs0Gbc")

            nc.gpsimd.dma_start(
                q0t[:].rearrange("p (i m) -> p i m", m=MS),
                w0t_in[:].rearrange("(i p) m -> p i m", p=128))
            nc.gpsimd.dma_start(
                q1t[:].rearrange("p (k d) -> p k d", d=D),
                w1t_in[:].rearrange("(k p) d -> p k d", p=128))
            nc.gpsimd.dma_start(
                q1n[:].rearrange("p (i m) -> p i m", m=MS),
                w1n_in[:].rearrange("(i p) m -> p i m", p=128))
            nc.sync.dma_start(bp0[:], b0c_in[:])
            nc.sync.dma_start(bk1[:], b1d8_in[:])
            nc.gpsimd.dma_start(ident[:], ident_in[:])
            nc.vector.memset(ones_col[:], 1.0)
            nc.vector.memset(ones_row[:], 1.0)
            nc.scalar.copy(bk1bf[:], bk1[:])
            nc.sync.dma_start(scal[:], scal_in[:])
            for j in range(NCH):
                for dst, col in ((cbc, SC_C), (cnbc, SC_CN), (fbc, SC_F),
                                 (negsbc, SC_NEGS), (negs0bc, SC_NEGS0),
                                 (negs0pbc, SC_NEGS0P), (negs0Gbc, SC_NEGS0G)):
                    nc.gpsimd.partition_broadcast(
                        dst[:, j:j + 1], scal[0:1, j * 8 + col:j * 8 + col + 1])

            # ---------------- helpers ----------------
            def load_x(j):
                xb = xp.tile([128, KT * D], BF, name=f"xb{j}", tag="xb")
                for k in range(KT):
                    nc.sync.dma_start(xb[:, k * D:(k + 1) * D],
                                      xr8_in[j, k * 128:(k + 1) * 128, :])
                xT = xp.tile([128, KD * T], BF, name=f"xT{j}", tag="xT")
                nc.sync.dma_start(
                    xT[:].rearrange("p (i t) -> p i t", t=T),
                    xt_in[j].rearrange("(i p) t -> p i t", p=128))
                return xb, xT

            def mmT(pt_slice, src_slice):
                nc.tensor.transpose(pt_slice, src_slice, ident[:])

            def mm1_full(xT, j, pfx, stop=True, zold_tag=False):
                """layer-1 q-space psum [m_s, KM*T] packed, under current q0t."""
                pool = psZ if zold_tag else psB
                pt = pool.tile([128, KM * T], F32, name=f"z{pfx}_{j}",
                               tag="psZ" if zold_tag else "psB")
                for a in range(KM):
                    for i in range(KD):
                        nc.tensor.matmul(
                            pt[:, a * T:(a + 1) * T],
                            q0t[:, i * MS + a * 128:i * MS + (a + 1) * 128],
                            xT[:, i * T:(i + 1) * T],
                            start=(i == 0), stop=(stop and i == KD - 1))
                return pt

            def silu_evac(zpt, j, cnc, want_dsilu, pfx):
                hT = (ap if want_dsilu else tp).tile(
                    [128, KM * T], BF, name=f"h{pfx}_{j}", tag=f"h{pfx}")
                for a in range(KM):
                    nc.scalar.activation(hT[:, a * T:(a + 1) * T],
                                         zpt[:, a * T:(a + 1) * T],
                                         AF.Silu, bias=bp0[:, a:a + 1],
                                         scale=cnc)
                if not want_dsilu:
                    return hT, None, None
                hpT = ap.tile([128, KM * T], BF, name=f"hp{pfx}_{j}", tag="hp1")

                def emit_dsilu():
                    for a in range(KM):
                        nc.scalar.activation(hpT[:, a * T:(a + 1) * T],
                                             zpt[:, a * T:(a + 1) * T],
                                             AF.Derivative_silu,
                                             bias=bp0[:, a:a + 1], scale=cnc)
                return hT, hpT, emit_dsilu

            def mm2_R(hT, j, pfx, evac, with_bias=False):
                """R-form layer 2 partial: psums [t',512], evac(k, n, pt) inline."""
                for k in range(KT):
                    for n in range(NN):
                        pt = psA.tile([128, 512], F32,
                                      name=f"p{pfx}_{j}_{k}_{n}", tag="psA")
                        for a in range(KM):
                            nc.tensor.matmul(
                                pt[:],
                                hT[:, a * T + k * 128:a * T + (k + 1) * 128],
                                q1t[:, a * D + n * 512:a * D + (n + 1) * 512],
                                start=(a == 0),
                                stop=(a == KM - 1 and not with_bias))
                        if with_bias:
                            nc.tensor.matmul(
                                pt[:], ones_row[:],
                                bk1bf[0:1, n * 512:(n + 1) * 512],
                                start=False, stop=True)
                        evac(k, n, pt)

            def evac_pred(k, n, pt, dst, csc, xb):
                # dst = c*psum - x/8 -> AR sums to dpred
                sl = slice(k * D + n * 512, k * D + (n + 1) * 512)
                nc.vector.scalar_tensor_tensor(dst[:, sl], pt[:], csc,
                                               xb[:, sl], ALU.mult,
                                               ALU.subtract)

            def send_ar(predp, j):
                arin = dr.tile([T, D], FP8, name=f"arin{j}", tag="arin")
                for k in range(KT):
                    for h in range(2):
                        eng = nc.sync if h == 0 else nc.scalar
                        eng.dma_start(
                            arin[k * 128:(k + 1) * 128,
                                 h * 1024:(h + 1) * 1024],
                            predp[:, k * D + h * 1024:k * D + (h + 1) * 1024])
                if ar:
                    arout = dr.tile([T, D], FP8, name=f"arout{j}", tag="arout",
                                    addr_space="Shared")
                    nc.gpsimd.collective_compute(
                        "AllReduce", ALU.add, replica_groups=[list(range(NCORES))],
                        ins=[arin.opt()], outs=[arout.opt()])
                else:
                    arout = arin
                return arout

            def h1R_T(h1T, j):
                """batched [t,m] form of h via matmul-transposes."""
                h1R = ap.tile([128, KT * MS], BF, name=f"h1R{j}", tag="h1R")
                pt = psT.tile([128, 1024], BF, name=f"Th{j}", tag="psT")
                for k in range(KT):
                    for a in range(KM):
                        mmT(pt[:, (k * KM + a) * 128:(k * KM + a + 1) * 128],
                            h1T[:, a * T + k * 128:a * T + (k + 1) * 128])
                nc.scalar.copy(h1R[:], pt[:, 0:KT * MS])
                return h1R

            def gram_G(xTa, xTb, jn):
                """Gs = negs0G_{jn} * (x_a x_b^T): [t(a) 2-tiles, t'(b)] bf16."""
                Gs = ap.tile([128, KT * T], BF, name=f"Gs{jn}", tag="Gs")
                pt = psB.tile([128, KT * T], F32, name=f"G{jn}", tag="psB")
                for k in range(KT):
                    for i in range(KD):
                        nc.tensor.matmul(
                            pt[:, k * T:(k + 1) * T],
                            xTa[:, i * T + k * 128:i * T + (k + 1) * 128],
                            xTb[:, i * T:(i + 1) * T],
                            start=(i == 0), stop=(i == KD - 1))
                nc.vector.tensor_scalar_mul(Gs[:], pt[:], negs0Gbc[:, jn:jn + 1])
                return Gs

            # ---------------- prologue: chunk 0 pred under P_0 ----------------
            xb_c, xT_c = load_x(0)
            xb_n, xT_n = load_x(1)
            c0 = cbc[:, 0:1]
            z0 = mm1_full(xT_c, 0, "1")
            h1T_c, hp1T_c, dsilu0 = silu_evac(z0, 0, c0, True, "1")
            predp = ap.tile([128, KT * D], FP8, name="predp0", tag="predp")
            mm2_R(h1T_c, 0, "p",
                  lambda k, n, pt: evac_pred(k, n, pt, predp, c0, xb_c),
                  with_bias=True)
            arout = send_ar(predp, 0)
            dsilu0()
            # ---- shadow of AR_0 ----
            h1R_c = h1R_T(h1T_c, 0)
            xb_p, xT_p = load_x(2)
            Gs_c = gram_G(xT_c, xT_n, 0)          # for crit_0's correction
            zold = mm1_full(xT_n, 1, "z", stop=False, zold_tag=True)

            # ---------------- main loop ----------------
            for j in range(NCH):
                last = (j == NCH - 1)
                cnj = cnbc[:, j:j + 1]
                fj = fbc[:, j:j + 1]
                ngj = negsbc[:, j:j + 1]
                ng0j = negs0bc[:, j:j + 1]

                # ======== critical section: consume AR_j ========
                dpF8 = tp.tile([128, KT * D], FP8, name=f"dpF8{j}", tag="dpF8")
                for k in range(KT):
                    nc.sync.dma_start(dpF8[:, k * D:(k + 1) * D],
                                      arout[k * 128:(k + 1) * 128, :])
                dpR = tp.tile([128, KT * D], BF, name=f"dpR{j}", tag="dpR")
                for k in range(KT):
                    nc.vector.tensor_copy(dpR[:, k * D:(k + 1) * D],
                                          dpF8[:, k * D:(k + 1) * D])
                nc.scalar.activation(junk[:], scal[0:1, 0:1], AF.Silu)

                # dpT via batched matmul-transposes (4 groups of 8)
                dpT = tp.tile([128, KD * T], BF, name=f"dpT{j}", tag="dpT")
                for g in range(4):
                    pt = psT.tile([128, 1024], BF, name=f"TdP{j}_{g}", tag="psT")
                    for x in range(8):
                        s = g * 8 + x
                        i, k = s // KT, s % KT
                        mmT(pt[:, x * 128:(x + 1) * 128],
                            dpR[:, k * D + i * 128:k * D + (i + 1) * 128])
                    eng = nc.vector if g % 2 else nc.scalar
                    if eng is nc.scalar:
                        nc.scalar.copy(dpT[:, g * 1024:(g + 1) * 1024], pt[:])
                    else:
                        eng.tensor_copy(dpT[:, g * 1024:(g + 1) * 1024], pt[:])

                # dhT = W1 dpred^T [m,t] ; dzT = dhT * hp
                dzT = tp.tile([128, KM * T], BF, name=f"dzT{j}", tag="dzT")
                pt = psB.tile([128, KM * T], F32, name=f"dh{j}", tag="psB")
                for a in range(KM):
                    for i in range(KD):
                        nc.tensor.matmul(
                            pt[:, a * T:(a + 1) * T],
                            q1n[:, i * MS + a * 128:i * MS + (a + 1) * 128],
                            dpT[:, i * T:(i + 1) * T],
                            start=(i == 0), stop=(i == KD - 1))
                nc.vector.tensor_tensor(dzT[:], pt[:], hp1T_c[:], ALU.mult)
                # dzR via batched matmul-transpose (one group of 4)
                dzR = tp.tile([128, KT * MS], BF, name=f"dzR{j}", tag="dzR")
                pt = psT.tile([128, 1024], BF, name=f"Tdz{j}", tag="psT")
                for k in range(KT):
                    for a in range(KM):
                        mmT(pt[:, (k * KM + a) * 128:(k * KM + a + 1) * 128],
                            dzT[:, a * T + k * 128:a * T + (k + 1) * 128])
                nc.vector.tensor_copy(dzR[:], pt[:, 0:KT * MS])

                # gb0 -> bp0 ; gb1 -> bk1 (both needed before silu / mm2 bias)
                nc.vector.tensor_scalar_mul(bp0[:], bp0[:], fj)
                ng0pj = negs0pbc[:, j:j + 1]
                gb0r = tp.tile([128, KM], F32, name=f"gb0_{j}", tag="gb0")
                nc.vector.reduce_sum(gb0r[:],
                                     dzT[:].rearrange("p (a t) -> p a t", t=T),
                                     axis=mybir.AxisListType.X)
                nc.vector.scalar_tensor_tensor(bp0[:], gb0r[:], ng0pj,
                                               bp0[:], ALU.mult, ALU.add)
                if not last:
                    # correction: zold += Gs^T-contract dz ; then silu
                    for a in range(KM):
                        for k in range(KT):
                            nc.tensor.matmul(
                                zold[:, a * T:(a + 1) * T],
                                dzR[:, k * MS + a * 128:k * MS + (a + 1) * 128],
                                Gs_c[:, k * T:(k + 1) * T],
                                start=False, stop=(k == KT - 1))
                    h1T_n, hp1T_n, dsilu_n = silu_evac(zold, j + 1, cnj, True, "1")
                else:
                    h1T_n = hp1T_n = dsilu_n = None
                # gW1 -> q1t update (PE) while dzR evacuates
                for a in range(KM):
                    for n in range(NN):
                        pt = psA.tile([128, 512], F32, name=f"g1_{j}_{a}_{n}",
                                      tag="psA")
                        for k in range(KT):
                            nc.tensor.matmul(
                                pt[:],
                                h1R_c[:, k * MS + a * 128:k * MS + (a + 1) * 128],
                                dpR[:, k * D + n * 512:k * D + (n + 1) * 512],
                                start=(k == 0), stop=(k == KT - 1))
                        sl = slice(a * D + n * 512, a * D + (n + 1) * 512)
                        if n % 2 == 0:
                            nc.vector.scalar_tensor_tensor(q1t[:, sl], pt[:], ngj,
                                                           q1t[:, sl], ALU.mult,
                                                           ALU.add)
                        else:
                            gq = tp.tile([128, 512], BF, name=f"gq1_{j}_{a}_{n}",
                                         tag="gq")
                            nc.scalar.mul(gq[:], pt[:], ngj)
                            nc.gpsimd.tensor_tensor(q1t[:, sl], q1t[:, sl], gq[:],
                                                    ALU.add)

                for n in range(NN):
                    pt = psD.tile([1, 512], F32, name=f"gb1_{j}_{n}", tag="psDr")
                    for k in range(KT):
                        nc.tensor.matmul(
                            pt[:], ones_col[:],
                            dpR[:, k * D + n * 512:k * D + (n + 1) * 512],
                            start=(k == 0), stop=(k == KT - 1))
                    sl = slice(n * 512, (n + 1) * 512)
                    nc.vector.scalar_tensor_tensor(
                        bk1[0:1, sl], pt[:],
                        scal[0:1, j * 8 + SC_NEGS8:j * 8 + SC_NEGS8 + 1],
                        bk1[0:1, sl], ALU.mult, ALU.add)
                nc.sync.dma_start(b1out[j:j + 1, :], bk1[:])
                nc.scalar.copy(bk1bf[:], bk1[:])

                if not last:
                    predp = ap.tile([128, KT * D], FP8, name=f"predp{j + 1}",
                                    tag="predp")
                    pp = predp
                    xbn = xb_n
                    mm2_R(h1T_n, j + 1, "p",
                          lambda k, n, pt: evac_pred(k, n, pt, pp, cnj, xbn),
                          with_bias=True)
                    arout = send_ar(predp, j + 1)
                    dsilu_n()

                # ======== shadow of AR_{j+1} ========
                # gW0 -> q0t update (paired psums; consumed by zOLD below)
                for i2 in range(KD // 2):
                    pt = psA.tile([128, 512], F32, name=f"g0_{j}_{i2}", tag="psA")
                    for ii in range(2):
                        i = i2 * 2 + ii
                        for k in range(KT):
                            nc.tensor.matmul(
                                pt[:, ii * MS:(ii + 1) * MS],
                                xb_c[:, k * D + i * 128:k * D + (i + 1) * 128],
                                dzR[:, k * MS:(k + 1) * MS],
                                start=(k == 0), stop=(k == KT - 1))
                    sl = slice(i2 * 512, (i2 + 1) * 512)
                    nc.vector.scalar_tensor_tensor(q0t[:, sl], pt[:], ng0j,
                                                   q0t[:, sl], ALU.mult, ALU.add)

                # out_j forward under P_{j+1} (unscaled Q-space partial)
                zo = mm1_full(xT_c, j, "2")
                h2T, _, _ = silu_evac(zo, j, cnj, False, "2")
                outsb = tp.tile([128, KT * D], BF, name=f"o{j}", tag="outsb")

                def evac_out(k, n, pt, dst=outsb):
                    sl = slice(k * D + n * 512, k * D + (n + 1) * 512)
                    if n % 2 == 0:
                        nc.vector.tensor_copy(dst[:, sl], pt[:])
                    else:
                        nc.scalar.copy(dst[:, sl], pt[:])

                mm2_R(h2T, j, "o", evac_out)
                for k in range(KT):
                    nc.sync.dma_start(outq[j, k * 128:(k + 1) * 128, :],
                                      outsb[:, k * D:(k + 1) * D])

                if not last:
                    # gW1n -> q1n update (vector-only evac: gpsimd may be
                    # blocked on the AR wait)
                    for i2 in range(KD // 2):
                        pt = psA.tile([128, 512], F32, name=f"g1n_{j}_{i2}",
                                      tag="psA")
                        for ii in range(2):
                            i = i2 * 2 + ii
                            for k in range(KT):
                                nc.tensor.matmul(
                                    pt[:, ii * MS:(ii + 1) * MS],
                                    dpR[:, k * D + i * 128:k * D + (i + 1) * 128],
                                    h1R_c[:, k * MS:(k + 1) * MS],
                                    start=(k == 0), stop=(k == KT - 1))
                        sl = slice(i2 * 512, (i2 + 1) * 512)
                        nc.vector.scalar_tensor_tensor(q1n[:, sl], pt[:], ngj,
                                                       q1n[:, sl], ALU.mult,
                                                       ALU.add)

                    h1R_n = h1R_T(h1T_n, j + 1)
                    if j + 2 < NCH:
                        # prepare next crit's pred basis: G gram + zOLD pass
                        Gs_n = gram_G(xT_n, xT_p, j + 1)
                        zold = mm1_full(xT_p, j + 2, "z", stop=False, zold_tag=True)
                        xb_f, xT_f = load_x(j + 3) if j + 3 < NCH else (None, None)
                        xb_c, xT_c = xb_n, xT_n
                        xb_n, xT_n = xb_p, xT_p
                        if xb_f is not None:
                            xb_p, xT_p = xb_f, xT_f
                        Gs_c = Gs_n
                    else:
                        xb_c, xT_c = xb_n, xT_n
                    h1T_c, hp1T_c, h1R_c = h1T_n, hp1T_n, h1R_n
    nc.compile()
    return nc


_NC_CACHE = None


def _get_nc():
    global _NC_CACHE
    if _NC_CACHE is None:
        _NC_CACHE = build()
    return _NC_CACHE


def _sigmoid(v):
    return 1.0 / (1.0 + np.exp(-v))


def host_prep(x, W0, b0, W1, b1, lr_w, lr_b, fg_w, fg_b):
    """Host-side: layouts, pre-transposed x, gate scalar schedule."""
    import ml_dtypes
    bf16 = ml_dtypes.bfloat16
    x = np.asarray(x, np.float32)
    # chunk layouts: xr8 [NCH, T, D] = x/8, xt [NCH, D, T] (token t = b*C + c)
    xch = np.transpose(x.reshape(B, NCH, C, D), (1, 0, 2, 3)).reshape(NCH, T, D)
    xr8 = np.ascontiguousarray(xch / 8.0).astype(bf16)
    xt = np.ascontiguousarray(np.transpose(xch, (0, 2, 1))).astype(bf16)

    lr_wv = np.asarray(lr_w, np.float32)[0]
    fg_wv = np.asarray(fg_w, np.float32)[0]
    lr_bv = float(np.asarray(lr_b, np.float32).reshape(-1)[0])
    fg_bv = float(np.asarray(fg_b, np.float32).reshape(-1)[0])
    scal = np.zeros((NCH, 8), np.float32)
    c = 1.0
    for j in range(NCH):
        ch = xch[j]                                   # (T, D)
        lsum = _sigmoid(ch @ lr_wv + lr_bv).sum()
        fparts = _sigmoid(ch.reshape(B, C, D).mean(axis=1) @ fg_wv + fg_bv)
        f = float(fparts.mean())
        cn = c * f
        negs = -LR_MEMORY * 2.0 * float(lsum) / (T * T * D) / cn
        # negs0 x8 (gW0 streams x/8); negs0G = negs*c (zOLD correction, x
        # unscaled in the gram matrix)
        scal[j] = [c, cn, f, negs, negs * c * 8.0, negs / 8.0, negs * c * cn,
                   negs * c]
        c = cn
    return xr8, xt, scal


def make_in_maps(x, W0, b0, W1, b1, lr_w, lr_b, fg_w, fg_b):
    xr8, xt, scal = host_prep(x, W0, b0, W1, b1, lr_w, lr_b, fg_w, fg_b)
    W0 = np.asarray(W0, np.float32)
    W1 = np.asarray(W1, np.float32)
    b0v = np.asarray(b0, np.float32)
    b1v = np.asarray(b1, np.float32)
    ident = np.eye(128, dtype=np.float32)
    in_maps = []
    for s in range(NCORES):
        sl = slice(s * MS, (s + 1) * MS)
        b0c = np.ascontiguousarray(b0v[sl].reshape(KM, 128).T)   # [128, KM]
        in_maps.append({
            "xr8": xr8,
            "xt": xt,
            "w0t": np.ascontiguousarray(W0[sl, :].T),
            "w1t": np.ascontiguousarray(W1[:, sl].T),
            "w1n": np.ascontiguousarray(W1[:, sl]),
            "b0c": b0c,
            "b1d8": np.ascontiguousarray((b1v / 8.0).reshape(1, D)),
            "scal": np.ascontiguousarray(scal.reshape(1, NCH * 8)),
            "ident": ident,
        })
    return in_maps


def run(inputs, **kw):
    nc = _get_nc()
    in_maps = make_in_maps(**inputs)
    res = run_bass_kernel_spmd(nc, in_maps, core_ids=list(range(NCORES)), **kw)
    scal = in_maps[0]["scal"].reshape(NCH, 8)
    outq = np.zeros((NCH, T, D), np.float32)
    for r in res.results:
        outq += np.asarray(r["outq"], dtype=np.float32)
    b1rows = res.results[0]["b1out"]                  # [NCH, D] = qb1/8 rows
    cn = scal[:, SC_CN].reshape(NCH, 1, 1)
    outq = cn * outq + (cn * 8.0) * b1rows.reshape(NCH, 1, D)
    out = np.ascontiguousarray(
        np.transpose(outq.reshape(NCH, B, C, D), (1, 0, 2, 3))).reshape(B, L, D)
    return out, res


def kernel(**inputs) -> np.ndarray:
    out, _ = run(inputs)
    return out
